# revision 1
# baseline (speedup 1.0000x reference)
"""Causal attention layer (B=4, N=2048, C=1024, H=16, D=64) on 8 TRN2 NeuronCores.

Sharding: core c -> (batch b = c//2, head-group g = c%2 of 8 heads).
Per core, for its (b, g):
  qkvT  = wqkvT_g.T-contract(x_b)      q,k transposed [o,n]; v transposed then
                                       DMA-transposed to [kn, 1|v] (ones col first)
  S_T   = kT.T @ qT                    pre-transposed scores [kn, qn], exp -> P_T bf16
  oT    = [1|v].T @ P_T                row 0 = softmax denominators, rows 1:65 = out.T
  attn_outT = oT[1:65] * bcast(1/oT[0])  (recip_approx_fast + gpsimd partition_broadcast)
  out_part  = attn_outT.T-contract(projT_g)
Host sums the two head-group partials per batch and adds proj_b.

The emission stream is software-pipelined: only the first v o-tile and pair-0
q/k run up front; all later v o-tiles, v_ext transposes, q/k projections, and
the output projection are queued as fillers and interleaved into the attention
stream so the PE stays dense (HAM-warm) while ScalarE runs exp.
"""
import sys

sys.path.insert(0, "/opt/trn_rl_repo")

import numpy as np

import concourse.bass as bass  # noqa: F401
import concourse.tile as tile
from concourse import bacc, mybir
from concourse.bass_utils import run_bass_kernel_spmd

F32 = mybir.dt.float32
F32R = mybir.dt.float32r
BF16 = mybir.dt.bfloat16
EXP = mybir.ActivationFunctionType.Exp

B, N, C, H, D = 4, 2048, 1024, 16, 64
G = 8            # heads per core
GC = G * D       # 512 channels per core
NT = N // 128    # 16 row tiles
NS = N // 512    # 4 row supers
CK = C // 128    # 8 contraction chunks

_cache = {}


def _build_nc():
    from contextlib import ExitStack

    nc = bacc.Bacc("TRN2", target_bir_lowering=False, debug=False)

    xT_d = nc.dram_tensor("xT", [C, N], F32R, kind="ExternalInput")
    wqkvT_d = nc.dram_tensor("wqkvT", [C, 3 * GC], F32R, kind="ExternalInput")
    projT_d = nc.dram_tensor("projT", [GC, C], BF16, kind="ExternalInput")
    tril_d = nc.dram_tensor("tril", [128, 128], BF16, kind="ExternalInput")
    ident_d = nc.dram_tensor("ident", [128, 128], BF16, kind="ExternalInput")
    onesb_d = nc.dram_tensor("onesb", [128, NT], BF16, kind="ExternalInput")
    out_d = nc.dram_tensor("out", [N, C], F32, kind="ExternalOutput")

    with tile.TileContext(nc) as tc:
        with ExitStack() as ctx:
            consts = ctx.enter_context(tc.tile_pool(name="consts", bufs=1))
            qk_pool = ctx.enter_context(tc.tile_pool(name="qk", bufs=4))
            vext_pool = ctx.enter_context(tc.tile_pool(name="vext", bufs=1))
            vT_pool = ctx.enter_context(tc.tile_pool(name="vT", bufs=2))
            w_pool = ctx.enter_context(tc.tile_pool(name="wA", bufs=3))
            xT_pool = ctx.enter_context(tc.tile_pool(name="xT", bufs=1))
            rf_pool = ctx.enter_context(tc.tile_pool(name="rf", bufs=2))
            bcs_pool = ctx.enter_context(tc.tile_pool(name="bcs", bufs=2))
            tmp_pool = ctx.enter_context(tc.tile_pool(name="tmp", bufs=2))
            ob_pool = ctx.enter_context(tc.tile_pool(name="ob", bufs=2))
            pj_pool = ctx.enter_context(tc.tile_pool(name="pj", bufs=1))
            psA = ctx.enter_context(tc.tile_pool(name="psA", bufs=2, space="PSUM"))

            tril_sb = consts.tile([128, 128], BF16)
            nc.sync.dma_start(tril_sb[:], tril_d[:])
            ident_sb = consts.tile([128, 128], BF16)
            nc.sync.dma_start(ident_sb[:], ident_d[:])

            early_w = {}

            def load_w(ot):
                if ot in early_w:
                    return early_w.pop(ot)
                wt = w_pool.tile([128, CK, 128], F32R, tag="wt", name=f"wt{ot}")
                src = wqkvT_d[:, 128 * ot:128 * (ot + 1)].rearrange(
                    "(cc p) o -> p cc o", p=128
                )
                nc.sync.dma_start(wt[:], src)
                return wt

            early_w.update({8: load_w(8), 0: load_w(0), 4: load_w(4)})

            v_ext = [vext_pool.tile([128, NT * 128], BF16, tag=f"ve{h}", name=f"ve{h}")
                     for h in range(G)]
            ve3 = [v.rearrange("p (n e) -> p n e", e=128) for v in v_ext]
            for h in range(G):
                nc.sync.dma_start(ve3[h][:, :, 0:1], onesb_d[:, :, None])

            xs = []
            for cc in range(CK):
                t = xT_pool.tile([128, N], F32R, tag=f"x{cc}", name=f"x{cc}")
                nc.sync.dma_start(t[:], xT_d[128 * cc:128 * (cc + 1), :])
                xs.append(t)

            pj_sb = [pj_pool.tile([128, C], BF16, tag=f"pj{i}", name=f"pj{i}")
                     for i in range(4)]
            for ac in range(4):
                nc.sync.dma_start(pj_sb[ac][:], projT_d[128 * ac:128 * (ac + 1), :])

            def qkv_quarter(wt, sup):
                psq = psA.tile([128, 512], F32, tag="qa", name="psq")
                for cc in range(CK):
                    nc.tensor.matmul(
                        psq[:],
                        wt[:, cc, :],
                        xs[cc][:, 512 * sup:512 * (sup + 1)],
                        start=(cc == 0),
                        stop=(cc == CK - 1),
                    )
                return psq

            tr_insts = [[None] * NT for _ in range(G)]

            # ------- step builders (emitted inline or queued as fillers) --------
            def v_steps(vp):
                """v o-tile vp -> vT (bf16) -> DMA-transpose into v_ext[2vp,2vp+1]."""
                vt = vT_pool.tile([128, N], BF16, tag="vt", name=f"vt{vp}")
                cps = [None] * NS
                steps = [("w", lambda vp=vp: load_w(8 + vp))]
                for sup in range(NS):
                    def _mms(wt, vt=vt, sup=sup):
                        psq = qkv_quarter(wt, sup)
                        cps[sup] = nc.vector.tensor_copy(
                            vt[:, 512 * sup:512 * (sup + 1)], psq[:]
                        )
                    steps.append(("q", _mms))
                for nt in range(NT):
                    def _tr(vt=vt, vp=vp, nt=nt):
                        tp = psA.tile([128, 128], BF16, tag="qa", name="tp")
                        nc.tensor.transpose(
                            tp[:], vt[:, 128 * nt:128 * (nt + 1)], ident_sb[:]
                        )
                        nc.vector.tensor_copy(
                            ve3[2 * vp][:, nt, 64:128], tp[:, 0:64]
                        )
                        nc.vector.tensor_copy(
                            ve3[2 * vp + 1][:, nt, 64:128], tp[:, 64:128]
                        )
                    steps.append(("p", _tr))
                return steps

            def qk_steps(p, interleave=False):
                qT = qk_pool.tile([128, N], F32R, tag="qk", name=f"q{p}")
                kT = qk_pool.tile([128, N], F32R, tag="qk", name=f"k{p}")
                if not interleave:
                    steps = []
                    for dst, ot in ((qT, p), (kT, 4 + p)):
                        steps.append(("w", lambda ot=ot: load_w(ot)))
                        for sup in range(NS):
                            def _mms(wt, dst=dst, sup=sup):
                                psq = qkv_quarter(wt, sup)
                                nc.vector.tensor_copy(
                                    dst[:, 512 * sup:512 * (sup + 1)], psq[:]
                                )
                            steps.append(("q", _mms))
                    return qT, kT, steps
                wts = {}
                steps = [
                    ("w", lambda: wts.setdefault("q", load_w(p))),
                    ("w", lambda: wts.setdefault("k", load_w(4 + p))),
                ]
                for sup in range(NS):
                    for key, dst in (("q", qT), ("k", kT)):
                        def _mms(_wt, key=key, dst=dst, sup=sup):
                            psq = qkv_quarter(wts[key], sup)
                            nc.vector.tensor_copy(
                                dst[:, 512 * sup:512 * (sup + 1)], psq[:]
                            )
                        steps.append(("q", _mms))
                return qT, kT, steps

            attn_outT = None

            def proj_steps(s):
                steps = []
                for nt in range(4 * s, 4 * s + 4):
                    for oc in (0, 1):
                        def _pj(nt=nt, oc=oc):
                            pp = psA.tile([128, 512], F32, tag="qa", name="pp")
                            for ac in range(4):
                                nc.tensor.matmul(
                                    pp[:],
                                    attn_outT[ac][:, 128 * nt:128 * (nt + 1)],
                                    pj_sb[ac][:, 512 * oc:512 * (oc + 1)],
                                    start=(ac == 0),
                                    stop=(ac == 3),
                                )
                            ob = ob_pool.tile([128, 512], F32, tag="ob", name="ob")
                            nc.vector.tensor_copy(ob[:], pp[:])
                            nc.sync.dma_start(
                                out_d[128 * nt:128 * (nt + 1),
                                      512 * oc:512 * (oc + 1)],
                                ob[:],
                            )
                        steps.append(("p", _pj))
                return steps

            # filler machinery: fill(n) emits until n PE-carrying steps are out
            pending = []
            state = {"wt": None}

            def fill(n):
                done = 0
                while pending and done < n:
                    kind, fn = pending.pop(0)
                    if kind == "w":
                        state["wt"] = fn()
                    elif kind == "q":
                        fn(state["wt"])
                        done += 1
                    elif kind == "t":
                        fn()
                    else:
                        fn()
                        done += 1

            def fill_all():
                while pending:
                    fill(4)

            # ---------------- prologue: vp0 + pair-0 q/k ------------------------
            for kind, fn in v_steps(0):
                if kind == "w":
                    state["wt"] = fn()
                elif kind == "q":
                    fn(state["wt"])
                else:
                    fn()
            qT, kT, steps0 = qk_steps(0, interleave=True)
            pending.extend(steps0)

            # ---------------- attention pair loop (with fillers) ----------------
            with (
                tc.tile_pool(name="aoT", bufs=1) as aoT_pool,
                tc.tile_pool(name="pt", bufs=7) as pt_pool,
                tc.tile_pool(name="psS", bufs=2, space="PSUM") as psS,
                tc.tile_pool(name="psO", bufs=2, space="PSUM") as psO,
            ):
                attn_outT = [aoT_pool.tile([128, N], BF16, tag=f"ao{p}", name=f"ao{p}")
                             for p in range(4)]
                for p in range(4):
                    if p < 3:
                        pending.extend(v_steps(p + 1))
                        nq, nk_, nsteps = qk_steps(p + 1)
                        pending.extend(nsteps)

                    for s in range(NS):
                        if p == 0:
                            fill(2)
                        nkb = 4 * (s + 1)
                        pts = {0: [], 1: []}
                        for kg in range(nkb // 2):
                            for h in (0, 1):
                                fill(2)
                                hh = slice(64 * h, 64 * (h + 1))
                                S2 = psS.tile([128, 1024], F32, tag="s2", name="S2")
                                for j in (0, 1):
                                    k = 2 * kg + j
                                    nc.tensor.matmul(
                                        S2[:, 512 * j:512 * (j + 1)],
                                        kT[hh, 128 * k:128 * (k + 1)],
                                        qT[hh, 512 * s:512 * (s + 1)],
                                    )
                                P2 = pt_pool.tile([128, 1024], BF16, tag="pt", name="P2")
                                nc.scalar.activation(
                                    P2[:], S2[:], EXP, scale=float(D) ** -0.5
                                )
                                for j in (0, 1):
                                    k = 2 * kg + j
                                    if k >= 4 * s:
                                        ridx = k - 4 * s
                                        c0 = 512 * j
                                        if ridx > 0:
                                            nc.vector.memset(
                                                P2[:, c0:c0 + 128 * ridx], 0.0
                                            )
                                        nc.vector.tensor_mul(
                                            P2[:, c0 + 128 * ridx:c0 + 128 * (ridx + 1)],
                                            P2[:, c0 + 128 * ridx:c0 + 128 * (ridx + 1)],
                                            tril_sb[:],
                                        )
                                pts[h].append(P2)
                        for h in (0, 1):
                            hg = 2 * p + h
                            oT = psO.tile([128, 512], F32, tag="oT", name="oT")
                            for kg in range(nkb // 2):
                                fill(2)
                                for j in (0, 1):
                                    k = 2 * kg + j
                                    nc.tensor.matmul(
                                        oT[:],
                                        ve3[hg][:, k, :],
                                        pts[h][kg][:, 512 * j:512 * (j + 1)],
                                        start=(k == 0),
                                        stop=(k == nkb - 1),
                                    )
                            Rf = rf_pool.tile([1, 512], F32, tag="rf", name="Rf")
                            nc.vector.reciprocal_approx_fast(Rf[:], oT[0:1, :])
                            bcs = bcs_pool.tile([128, 512], F32, tag="bcs", name="bcs")
                            nc.gpsimd.partition_broadcast(bcs[:], Rf[:])
                            tmp = tmp_pool.tile([128, 512], BF16, tag="tmp", name="tmp")
                            nc.vector.tensor_mul(tmp[:], oT[:], bcs[:])
                            nc.sync.dma_start(
                                attn_outT[p][64 * h:64 * (h + 1),
                                             512 * s:512 * (s + 1)],
                                tmp[64:128, :],
                            )
                        if p == 3:
                            pending.extend(proj_steps(s))
                            fill(2)
                    if p < 3:
                        qT, kT = nq, nk_
                fill_all()

    nc.compile()
    return nc


def _tril_np():
    import ml_dtypes

    i = np.arange(128)[:, None]
    j = np.arange(128)[None, :]
    return (j >= i).astype(np.float32).astype(ml_dtypes.bfloat16)


def make_in_maps(x, qkv_w, proj_w):
    x = np.asarray(x, dtype=np.float32)
    qkv_w = np.asarray(qkv_w, dtype=np.float32)
    proj_w = np.asarray(proj_w, dtype=np.float32)
    tril = _tril_np()
    in_maps = []
    for c in range(8):
        b, g = c // 2, c % 2
        sl = slice(g * GC, (g + 1) * GC)
        wq, wk, wv = qkv_w[0:C][sl], qkv_w[C:2 * C][sl], qkv_w[2 * C:3 * C][sl]
        in_maps.append(
            {
                "xT": np.ascontiguousarray(x[b].T),
                "wqkvT": np.ascontiguousarray(np.concatenate([wq, wk, wv], 0).T),
                "projT": np.ascontiguousarray(proj_w[:, sl].T).astype(
                    __import__("ml_dtypes").bfloat16
                ),
                "tril": tril,
                "ident": np.eye(128, dtype=np.float32).astype(
                    __import__("ml_dtypes").bfloat16
                ),
                "onesb": np.ones((128, NT), dtype=np.float32).astype(
                    __import__("ml_dtypes").bfloat16
                ),
            }
        )
    return in_maps


def kernel(x, qkv_w, proj_w, proj_b):
    proj_b = np.asarray(proj_b, dtype=np.float32)

    if "nc" not in _cache:
        _cache["nc"] = _build_nc()
    nc = _cache["nc"]

    in_maps = make_in_maps(x, qkv_w, proj_w)
    res = run_bass_kernel_spmd(nc, in_maps, core_ids=list(range(8)))
    out = np.stack(
        [res.results[2 * b]["out"] + res.results[2 * b + 1]["out"] for b in range(B)], 0
    )
    return (out + proj_b[None, None, :]).astype(np.float32)



# revision 7
# speedup vs baseline: 1.1092x; 1.1092x over previous
"""Causal attention layer (B=4, N=2048, C=1024, H=16, D=64) on 8 TRN2 NeuronCores.

Sharding: core c -> (batch b = c//2, head-group g = c%2 of 8 heads).

Per core, all PE operands are bf16 (LDWEIGHTS is 100ns for bf16 vs 200ns for
fp32r, and bf16 avoids the fp32r 4x penalty for moving dims < 256):

  qkv   : per o-tile (128 channels) and q-megablock (1024 tokens), accumulate
          8x [128,128] w-chunks against x chunks -> psum [128,1024] -> sbuf.
  attn  : per (head, megablock): k-outer loop. S_k = kT-tile^T qT (ap shrinks
          near the diagonal), exp on ScalarE, tril-mask on DVE for diagonal
          tiles, AV accumulates [1|v]^T P into a single psum tile using
          suffix ranges (causality at 128-key granularity).
  proj  : flipped: stationary = proj chunk, moving = attn_outT -> output is
          TRANSPOSED [C, N]; host transposes back (host time is free).

The qkv/transpose/proj units are queued as single-matmul filler closures and
interleaved into the attention k-loop so the PE never waits on ScalarE exp.
"""
import sys

sys.path.insert(0, "/opt/trn_rl_repo")

import numpy as np

import concourse.bass as bass  # noqa: F401
import concourse.tile as tile
from concourse import bacc, mybir
from concourse.bass_utils import run_bass_kernel_spmd

F32 = mybir.dt.float32
BF16 = mybir.dt.bfloat16
EXP = mybir.ActivationFunctionType.Exp

B, N, C, H, D = 4, 2048, 1024, 16, 64
G = 8            # heads per core
GC = G * D       # 512 channels per core
NT = N // 128    # 16 k-tiles
CK = C // 128    # 8 contraction chunks
MB = 1024        # q-megablock width
SCALE = float(D) ** -0.5

_cache = {}


def _build_nc():
    from contextlib import ExitStack

    nc = bacc.Bacc("TRN2", target_bir_lowering=False, debug=False)

    xT_d = nc.dram_tensor("xT", [C, N], BF16, kind="ExternalInput")
    wqkvT_d = nc.dram_tensor("wqkvT", [C, 3 * GC], BF16, kind="ExternalInput")
    projT_d = nc.dram_tensor("projT", [GC, C], BF16, kind="ExternalInput")
    tril_d = nc.dram_tensor("tril", [128, 128], BF16, kind="ExternalInput")
    ident_d = nc.dram_tensor("ident", [128, 128], BF16, kind="ExternalInput")
    onesb_d = nc.dram_tensor("onesb", [128, NT], BF16, kind="ExternalInput")
    outT_d = nc.dram_tensor("outT", [C, N], F32, kind="ExternalOutput")

    with tile.TileContext(nc) as tc:
        with ExitStack() as ctx:
            consts = ctx.enter_context(tc.tile_pool(name="consts", bufs=1))
            wt_pool = ctx.enter_context(tc.tile_pool(name="wt", bufs=1))
            xs_pool = ctx.enter_context(tc.tile_pool(name="xs", bufs=1))
            qk_pool = ctx.enter_context(tc.tile_pool(name="qk", bufs=4))
            vT_pool = ctx.enter_context(tc.tile_pool(name="vT", bufs=2))
            vext_pool = ctx.enter_context(tc.tile_pool(name="vext", bufs=1))
            pt_pool = ctx.enter_context(tc.tile_pool(name="pt", bufs=4))
            aoT_pool = ctx.enter_context(tc.tile_pool(name="aoT", bufs=1))
            pj_pool = ctx.enter_context(tc.tile_pool(name="pj", bufs=1))
            os_pool = ctx.enter_context(tc.tile_pool(name="os", bufs=2))
            rf_pool = ctx.enter_context(tc.tile_pool(name="rf", bufs=2))
            bcs_pool = ctx.enter_context(tc.tile_pool(name="bcs", bufs=2))
            tmp_pool = ctx.enter_context(tc.tile_pool(name="tmp", bufs=2))
            ob_pool = ctx.enter_context(tc.tile_pool(name="ob", bufs=2))
            psS = ctx.enter_context(tc.tile_pool(name="psS", bufs=2, space="PSUM"))
            psO = ctx.enter_context(tc.tile_pool(name="psO", bufs=1, space="PSUM"))
            psF = ctx.enter_context(tc.tile_pool(name="psF", bufs=1, space="PSUM"))

            tril_sb = consts.tile([128, 128], BF16)
            nc.sync.dma_start(tril_sb[:], tril_d[:])
            ident_sb = consts.tile([128, 128], BF16)
            nc.sync.dma_start(ident_sb[:], ident_d[:])

            # all 12 qkv-weight o-tiles in one sbuf tensor [128, cc, 12*128]
            wt_all = wt_pool.tile([128, CK, 3 * GC], BF16, tag="wt", name="wt")

            def load_w(ot):
                src = wqkvT_d[:, 128 * ot:128 * (ot + 1)].rearrange(
                    "(cc p) o -> p cc o", p=128
                )
                nc.sync.dma_start(wt_all[:, :, 128 * ot:128 * (ot + 1)], src)

            # o-tiles: q pairs 0-3, k pairs 4-7, v pairs 8-11.
            # DMA order: what the prologue needs first.
            for ot in (8, 0, 4):
                load_w(ot)

            # x chunks, mb0 halves first (prologue computes mb0 qkv first)
            xs = xs_pool.tile([128, CK, N], BF16, tag="xs", name="xs")
            for mb in range(2):
                for cc in range(CK):
                    nc.sync.dma_start(
                        xs[:, cc, MB * mb:MB * (mb + 1)],
                        xT_d[128 * cc:128 * (cc + 1), MB * mb:MB * (mb + 1)],
                    )

            for ot in (9, 1, 5, 10, 2, 6, 11, 3, 7):
                load_w(ot)

            v_ext = [vext_pool.tile([128, NT, 128], BF16, tag=f"ve{h}",
                                    name=f"ve{h}")
                     for h in range(G)]
            for h in range(G):
                nc.sync.dma_start(v_ext[h][:, :, 0:1], onesb_d[:, :, None])

            pj_sb = pj_pool.tile([128, 4, C], BF16, tag="pj", name="pj")
            for gcc in range(4):
                nc.sync.dma_start(
                    pj_sb[:, gcc, :], projT_d[128 * gcc:128 * (gcc + 1), :]
                )

            aoT = [aoT_pool.tile([128, N], BF16, tag=f"ao{p}", name=f"ao{p}")
                   for p in range(4)]

            # ---------------- filler units (1 PE op per closure) -----------
            # PE matmul dst must stay inside one PSUM bank (512 f32 cols).
            def bank_pieces(c0, c1):
                pieces = []
                c = c0
                while c < c1:
                    e = min(c1, (c // 512 + 1) * 512)
                    pieces.append((c, e))
                    c = e
                return pieces

            def qkv_unit(ot, mb, dst, dst_dt_tag):
                """16 closures: accumulate psum [128,1024], copy to dst slice."""
                cell = {}
                steps = []
                for cc in range(CK):
                    for a, b in ((0, 512), (512, MB)):
                        def _mm(cc=cc, ot=ot, mb=mb, dst=dst, a=a, b=b):
                            if cc == 0 and a == 0:
                                cell["ps"] = psF.tile([128, MB], F32, tag="F",
                                                      name=f"ps{ot}_{mb}")
                            nc.tensor.matmul(
                                cell["ps"][:, a:b],
                                wt_all[:, cc, 128 * ot:128 * (ot + 1)],
                                xs[:, cc, MB * mb + a:MB * mb + b],
                                start=(cc == 0),
                                stop=(cc == CK - 1),
                            )
                            if cc == CK - 1 and b == MB:
                                nc.vector.tensor_copy(
                                    dst[:, MB * mb:MB * (mb + 1)], cell["ps"][:]
                                )
                        steps.append(_mm)
                return steps

            def tr_unit(vp, vt, nt):
                """1 closure: transpose one 128-col v tile into v_ext."""
                def _tr(vp=vp, vt=vt, nt=nt):
                    tp = psF.tile([128, 128], BF16, tag="F", name="tp")
                    nc.tensor.transpose(
                        tp[:], vt[:, 128 * nt:128 * (nt + 1)], ident_sb[:]
                    )
                    nc.vector.tensor_copy(
                        v_ext[2 * vp][:, nt, 64:128], tp[:, 0:64]
                    )
                    nc.vector.tensor_copy(
                        v_ext[2 * vp + 1][:, nt, 64:128], tp[:, 64:128]
                    )
                return [_tr]

            def proj_unit(co, mb):
                """4 closures: accumulate 4 gc-chunks, copy+DMA out (transposed)."""
                cell = {}
                steps = []
                for gcc in range(4):
                    for a, b in ((0, 512), (512, MB)):
                        def _mm(gcc=gcc, co=co, mb=mb, a=a, b=b):
                            if gcc == 0 and a == 0:
                                cell["ps"] = psF.tile([128, MB], F32, tag="F",
                                                      name=f"pp{co}_{mb}")
                            nc.tensor.matmul(
                                cell["ps"][:, a:b],
                                pj_sb[:, gcc, 128 * co:128 * (co + 1)],
                                aoT[gcc][:, MB * mb + a:MB * mb + b],
                                start=(gcc == 0),
                                stop=(gcc == 3),
                            )
                            if gcc == 3 and b == MB:
                                ob = ob_pool.tile([128, MB], F32, tag="ob",
                                                  name="ob")
                                nc.vector.tensor_copy(ob[:], cell["ps"][:])
                                nc.sync.dma_start(
                                    outT_d[128 * co:128 * (co + 1),
                                           MB * mb:MB * (mb + 1)],
                                    ob[:],
                                )
                        steps.append(_mm)
                return steps

            pending = []

            def fill(n):
                for _ in range(min(n, len(pending))):
                    pending.pop(0)()

            # ---------------- attention chain ------------------------------
            def chain(p, h, mb, qT, kT, fills_per_step):
                hg = 2 * p + h
                hh = slice(64 * h, 64 * (h + 1))
                kmax = 8 * (mb + 1)
                Ss = [None] * kmax
                Ps = [None] * kmax
                ws = [0] * kmax
                c0s = [0] * kmax

                def emit_s_exp(k):
                    qs_abs = max(MB * mb, 128 * k)
                    w = MB * (mb + 1) - qs_abs
                    ws[k] = w
                    c0s[k] = qs_abs - MB * mb
                    S = psS.tile([128, MB], F32, tag="S", name=f"S{k}")
                    for a, b in bank_pieces(0, w):
                        nc.tensor.matmul(
                            S[:, a:b],
                            kT[hh, 128 * k:128 * (k + 1)],
                            qT[hh, qs_abs + a:qs_abs + b],
                        )
                    P = pt_pool.tile([128, MB], BF16, tag="P", name=f"P{k}")
                    nc.scalar.activation(P[:, 0:w], S[:, 0:w], EXP, scale=SCALE)
                    if 128 * k >= MB * mb:
                        nc.vector.tensor_mul(P[:, 0:128], P[:, 0:128], tril_sb[:])
                    Ss[k] = S
                    Ps[k] = P

                oT = psO.tile([128, MB], F32, tag="O", name="oT")
                emit_s_exp(0)
                if kmax > 1:
                    emit_s_exp(1)
                for k in range(kmax):
                    fill(fills_per_step)
                    c0 = c0s[k]
                    for a, b in bank_pieces(c0, MB):
                        nc.tensor.matmul(
                            oT[:, a:b],
                            v_ext[hg][:, k, :],
                            Ps[k][:, a - c0:b - c0],
                            start=(k == 0),
                            stop=(k == kmax - 1),
                            skip_group_check=True,
                        )
                    if k + 2 < kmax:
                        emit_s_exp(k + 2)

                # normalize off-psum: copy out first so psO frees fast
                oS = os_pool.tile([128, MB], F32, tag="os", name="oS")
                nc.vector.tensor_copy(oS[:], oT[:])
                Rf = rf_pool.tile([1, MB], F32, tag="rf", name="Rf")
                nc.vector.reciprocal_approx_fast(Rf[:], oS[0:1, :])
                bcs = bcs_pool.tile([128, MB], F32, tag="bcs", name="bcs")
                nc.gpsimd.partition_broadcast(bcs[:], Rf[:])
                tmp = tmp_pool.tile([128, MB], BF16, tag="tmp", name="tmp")
                nc.vector.tensor_mul(tmp[64:128, :], oS[64:128, :], bcs[64:128, :])
                nc.sync.dma_start(
                    aoT[p][64 * h:64 * (h + 1), MB * mb:MB * (mb + 1)],
                    tmp[64:128, :],
                )

            # ---------------- prologue -------------------------------------
            qTs, kTs, vTs = {}, {}, {}
            for p in range(4):
                qTs[p] = qk_pool.tile([128, N], BF16, tag="qk", name=f"q{p}")
                kTs[p] = qk_pool.tile([128, N], BF16, tag="qk", name=f"k{p}")
            for p in range(4):
                vTs[p] = vT_pool.tile([128, N], BF16, tag="vt", name=f"v{p}")

            def emit(steps):
                for s in steps:
                    s()

            emit(qkv_unit(8, 0, vTs[0], "v"))          # v0 mb0
            emit(qkv_unit(0, 0, qTs[0], "q"))          # q0 mb0
            emit(qkv_unit(4, 0, kTs[0], "k"))          # k0 mb0
            for nt in range(4):
                emit(tr_unit(0, vTs[0], nt))

            # remainder of pair0's deps (36 steps), consumed early in pair0
            for nt in range(4, 8):
                pending += tr_unit(0, vTs[0], nt)
            pending += qkv_unit(8, 1, vTs[0], "v")     # v0 mb1
            pending += qkv_unit(0, 1, qTs[0], "q")     # q0 mb1
            pending += qkv_unit(4, 1, kTs[0], "k")     # k0 mb1
            for nt in range(8, 16):
                pending += tr_unit(0, vTs[0], nt)
            # pair p block (64 steps each) must fully drain during pair p-1
            for p in range(1, 4):
                pending += qkv_unit(8 + p, 0, vTs[p], "v")
                pending += qkv_unit(8 + p, 1, vTs[p], "v")
                pending += qkv_unit(p, 0, qTs[p], "q")
                pending += qkv_unit(p, 1, qTs[p], "q")
                for nt in range(NT):
                    pending += tr_unit(p, vTs[p], nt)
                pending += qkv_unit(4 + p, 0, kTs[p], "k")
                pending += qkv_unit(4 + p, 1, kTs[p], "k")

            # ---------------- pair loop ------------------------------------
            # fills/step: pair0 must drain 172 queued steps (its own tail 60 +
            # pair1's 112-step block); later pairs drain the next 112-block.
            FPS = {0: (6, 4), 1: (2, 2), 2: (2, 2), 3: (2, 4)}
            for p in range(4):
                for mb in (0, 1):
                    for h in (0, 1):
                        chain(p, h, mb, qTs[p], kTs[p], FPS[p][mb])
                    if p == 3:
                        for co in range(CK):
                            pending += proj_unit(co, mb)
            fill(len(pending))

    nc.compile()
    return nc


def _tril_np():
    import ml_dtypes

    i = np.arange(128)[:, None]
    j = np.arange(128)[None, :]
    return (j >= i).astype(np.float32).astype(ml_dtypes.bfloat16)


def make_in_maps(x, qkv_w, proj_w):
    import ml_dtypes

    bf16 = ml_dtypes.bfloat16
    x = np.asarray(x, dtype=np.float32)
    qkv_w = np.asarray(qkv_w, dtype=np.float32)
    proj_w = np.asarray(proj_w, dtype=np.float32)
    tril = _tril_np()
    ident = np.eye(128, dtype=np.float32).astype(bf16)
    onesb = np.ones((128, NT), dtype=np.float32).astype(bf16)
    in_maps = []
    for c in range(8):
        b, g = c // 2, c % 2
        sl = slice(g * GC, (g + 1) * GC)
        wq, wk, wv = qkv_w[0:C][sl], qkv_w[C:2 * C][sl], qkv_w[2 * C:3 * C][sl]
        in_maps.append(
            {
                "xT": np.ascontiguousarray(x[b].T).astype(bf16),
                "wqkvT": np.ascontiguousarray(
                    np.concatenate([wq, wk, wv], 0).T
                ).astype(bf16),
                "projT": np.ascontiguousarray(proj_w[:, sl].T).astype(bf16),
                "tril": tril,
                "ident": ident,
                "onesb": onesb,
            }
        )
    return in_maps


def kernel(x, qkv_w, proj_w, proj_b):
    proj_b = np.asarray(proj_b, dtype=np.float32)

    if "nc" not in _cache:
        _cache["nc"] = _build_nc()
    nc = _cache["nc"]

    in_maps = make_in_maps(x, qkv_w, proj_w)
    res = run_bass_kernel_spmd(nc, in_maps, core_ids=list(range(8)))
    out = np.stack(
        [
            (res.results[2 * b]["outT"] + res.results[2 * b + 1]["outT"]).T
            for b in range(B)
        ],
        0,
    )
    return (out + proj_b[None, None, :]).astype(np.float32)
